# revision 1
# baseline (speedup 1.0000x reference)
"""Single-head attention (no 1/sqrt(d) scaling) for Trainium2, 8 NeuronCores.

Problem: x [8, 2048, 768], W [2304, 768], b [2304]
    qkv = x @ W.T + b ; q,k,v = split(qkv)
    out = softmax(q @ k.T) @ v            -> [8, 2048, 768] fp32

Sharding: data-parallel over batch, one batch element per core. Inputs are
host-transposed (xT [768,2048], wT [768,2304]); the kernel emits out^T
[768, 2048] and the host transposes back during the gather.

All matmuls run in fp32r (full PE rate, ~1.5e-4 rel rounding — bf16 anywhere
upstream of the softmax would blow up the logit error, measured end-to-end
rel err stays ~9e-4).

Phase A (k/v projection), looped over 512-wide n-slices of x streaming
through double-buffered SBUF slots, W resident:
    kT = (x @ Wk.T + bk).T  transposed layout [h, n] (lhsT = W block, rhs =
         xT slice; per-partition bias via the ACT eviction). Resident.
    v  = x @ Wv.T + bv      natural layout (lhsT = xT block, rhs = W slice;
         bias via a partition-broadcast DVE add at eviction). Resident.
Phase B (attention) per 512-wide n-slice; no max subtraction (|logits| <~60
<< 88 so exp stays within fp32 range; denominators handled unnormalized):
    qT strip = (x @ Wq.T + bq).T  projected on the fly (no spill round-trip)
    S^T[m,n] = k qT       (PSUM, 6 accumulating matmuls per m-chunk)
    P = exp(S^T)          (ACT, rounds to fp32r)
    U^T += v_m^T @ P      (6 matmuls, accumulated over 16 m-chunks in 6 banks)
    r   += ones128 @ P    (replicated denominator, DVE-accumulated in SBUF)
    out^T slice = U^T * (1/r)  (DVE scale at eviction, DMA straight to DRAM)
The m-loop is software-pipelined (S/exp for chunk i issued ahead of r/U for
chunk i-1) so the PE never waits on the exp; S tiles double-buffer through
2 PSUM banks, U holds 6 banks.
"""

import contextlib

import numpy as np

import concourse.bacc as bacc
import concourse.mybir as mybir
import concourse.tile as tile
from concourse.bass_utils import run_bass_kernel_spmd

F32 = mybir.dt.float32
F32R = mybir.dt.float32r
AF = mybir.ActivationFunctionType
ALU = mybir.AluOpType

B, N, H = 8, 2048, 768
H3 = 3 * H
P = 128
ND = H // P      # 6 d-chunks
NM = N // P      # 16 m-chunks
SL = 512         # n-slice width (fp32 moving-operand max / one PSUM bank)
NSL = N // SL    # 4 n-slices


def build_nc(loop_iters=None, split=1, nm_eff=NM, nsl_eff=NSL, b_off=False, no_r=False, copy_exp=False, pv_bf16=False, STORE_GP=True, SPILL_GP=False):
    """Build the attention kernel. loop_iters wraps the whole body in an
    on-device For_i loop (benchmarking only — amortizes dispatch overhead).
    split=2 issues every N=512 matmul as two N=256 halves (same PSUM bank,
    one accumulation group) — empirically faster moving-operand streaming."""
    HS = SL // split  # matmul moving width
    nc = bacc.Bacc("TRN2", target_bir_lowering=False, debug=False)

    xT = nc.dram_tensor("xT", [H, N], F32R, kind="ExternalInput")
    wT = nc.dram_tensor("wT", [H, H3], F32R, kind="ExternalInput")
    bcol = nc.dram_tensor("bcol", [P, 2 * ND], F32, kind="ExternalInput")
    bvrow = nc.dram_tensor("bvrow", [1, H], F32, kind="ExternalInput")
    out = nc.dram_tensor("out", [H, N], F32, kind="ExternalOutput")  # transposed; host fixes layout


    def mm_group(psum, lhs_list, rhs_slicer, extra=None, split=1):
        """Accumulating matmul group into `psum` [P, SL-or-less wide].

        lhs_list: per-c stationary APs; rhs_slicer(c, lo, w): moving AP slice.
        extra: optional (lhsT, rhs_slicer) K=1 bias pair appended to the group.
        """
        width = psum.shape[-1]
        hw = width // split
        n = len(lhs_list)
        first, last = True, None
        steps = []
        for c in range(n):
            for h in range(split):
                steps.append(("mm", c, h))
        if extra is not None:
            for h in range(split):
                steps.append(("extra", 0, h))
        for idx, (kind, c, h) in enumerate(steps):
            stop = idx == len(steps) - 1
            lo = h * hw
            if kind == "mm":
                nc.tensor.matmul(
                    psum[:, lo : lo + hw], lhs_list[c], rhs_slicer(c, lo, hw),
                    start=(idx == 0), stop=stop,
                )
            else:
                elh, ers = extra
                nc.tensor.matmul(
                    psum[:, lo : lo + hw], elh, ers(0, lo, hw),
                    start=False, stop=stop,
                )

    with tile.TileContext(nc) as tc:
        with (
            tc.tile_pool(name="dram", bufs=1, space="DRAM") as dram,
            tc.tile_pool(name="const", bufs=1) as const,
            tc.tile_pool(name="keep", bufs=1) as keep,
            tc.For_i(0, loop_iters, 1) if loop_iters else contextlib.nullcontext(),
        ):
            bcol_sb = const.tile([P, 2 * ND], F32)
            nc.sync.dma_start(bcol_sb[:], bcol.ap())

            BF16 = mybir.dt.bfloat16
            pdt = BF16 if pv_bf16 else F32R
            ones128 = const.tile([P, P], pdt)  # lhsT for the replicated-r matmul
            ones_f32, ones_free = tc.tile([P, P], F32, name="ones_f32")
            nc.gpsimd.memset(ones_f32[:], 1.0)
            nc.scalar.copy(ones128[:], ones_f32[:])
            ones_free()

            # resident across phases
            ktsb = [keep.tile([P, N], F32R, name=f"kT{c}") for c in range(ND)]
            vsb = [keep.tile([P, H], pdt, name=f"v{ni}") for ni in range(NM)]

            with tc.tile_pool(name="xw_pool", bufs=1) as xw:
                # W resident. q/k sections as [128,128] h-slices so compute
                # unlocks at DMA-stream granularity; v as [128, 768].
                HH = H // 2
                wq = [
                    [xw.tile([P, HH], F32R, name=f"wq{c}_{h}") for h in range(2)]
                    for c in range(ND)
                ]

                def wslice(blks, c, hc):
                    half, col = divmod(hc * P, HH)
                    return blks[c][half][:, col : col + P]
                xwa = tc.alloc_tile_pool(name="xwa_pool", bufs=1)
                wk = [
                    [xwa.tile([P, HH], F32R, name=f"wk{c}_{h}") for h in range(2)]
                    for c in range(ND)
                ]
                wv = [xwa.tile([P, H], F32R, name=f"wv{c}") for c in range(ND)]
                # x slices stream through 2 slots per d-chunk; every load
                # allocates fresh tiles so the tag rotation stays consistent
                xts = {}

                def fresh_xt(s, phase):
                    tiles = [
                        xw.tile([P, SL], F32R, name=f"xt{phase}{c}_{s}",
                                tag=f"xt{c}", bufs=2)
                        for c in range(ND)
                    ]
                    for c in range(ND):
                        nc.sync.dma_start(
                            tiles[c][:],
                            xT.ap()[c * P : (c + 1) * P, s * SL : (s + 1) * SL],
                        )
                    xts[s] = tiles
                    return tiles

                def load_w_half(blks, lo, h):
                    for c in range(ND):
                        nc.sync.dma_start(
                            blks[c][h][:],
                            wT.ap()[c * P : (c + 1) * P, lo + h * HH : lo + (h + 1) * HH],
                        )

                bvb = xwa.tile([P, H], F32, name="bvb")
                nc.sync.dma_start(bvb[:1, :], bvrow.ap())
                nc.gpsimd.partition_broadcast(bvb[:], bvb[:1, :])

                # DMA order = compute-unlock order: phase A starts with the
                # k projection, so k weights + x slice 0 first; wq (only
                # needed in phase B) last.
                load_w_half(wk, H, 0)
                fresh_xt(0, "a")
                load_w_half(wk, H, 1)
                for c in range(ND):
                    nc.sync.dma_start(
                        wv[c][:], wT.ap()[c * P : (c + 1) * P, 2 * H : 3 * H]
                    )
                fresh_xt(1, "a")
                load_w_half(wq, 0, 0)
                load_w_half(wq, 0, 1)

                with (
                    tc.tile_pool(name="qkps", bufs=3, space="PSUM") as qkps,
                    tc.tile_pool(name="vps", bufs=2, space="PSUM") as vps,
                ):
                    for ns in range(NSL):
                        ssl = slice(ns * SL, (ns + 1) * SL)
                        if ns >= 1 and ns + 1 < NSL:
                            fresh_xt(ns + 1, "a")

                        # --- k projection for this slice (resident) ---
                        for hc in range(ND):
                            ps = qkps.tile([P, SL], F32, name="qkpsum", tag="qk")
                            mm_group(
                                ps, [wslice(wk, c, hc) for c in range(ND)],
                                lambda c, lo, w, _ns=ns: xts[_ns][c][:, lo : lo + w],
                                split=split,
                            )
                            nc.scalar.activation(
                                ktsb[hc][:, ssl], ps[:], AF.Identity,
                                bias=bcol_sb[:, ND + hc : ND + hc + 1],
                            )

                        # --- v projection for the 4 n-chunks of this slice ---
                        for ni in range(4 * ns, 4 * ns + 4):
                            lsl = slice((ni % NSL) * P, (ni % NSL) * P + P)
                            pa = vps.tile([P, SL], F32, name="pa", tag="pa")
                            pb = vps.tile([P, H - SL], F32, name="pb", tag="pb")
                            mm_group(
                                pa, [xts[ns][c][:, lsl] for c in range(ND)],
                                lambda c, lo, w: wv[c][:, lo : lo + w],
                                split=split,
                            )
                            mm_group(
                                pb, [xts[ns][c][:, lsl] for c in range(ND)],
                                lambda c, lo, w: wv[c][:, SL + lo : SL + lo + w],
                            )
                            nc.vector.tensor_tensor(
                                vsb[ni][:, 0:SL], pa[:], bvb[:, 0:SL], op=ALU.add
                            )
                            nc.vector.tensor_tensor(
                                vsb[ni][:, SL:H], pb[:], bvb[:, SL:H], op=ALU.add
                            )

                for s in range(min(2, nsl_eff)):
                    fresh_xt(s, "b")
                xwa.release()

                if b_off:
                    for c in range(ND):
                        nc.sync.dma_start(
                            out.ap()[c * P : (c + 1) * P, :], ktsb[c][:]
                        )
                # ---- Phase B: attention (software-pipelined m-loop) ----
                with (
                    contextlib.nullcontext() if b_off else contextlib.nullcontext(),
                    tc.tile_pool(name="qsb_pool", bufs=2) as qsb_pool,
                    tc.tile_pool(name="p_pool", bufs=4) as p_pool,
                    tc.tile_pool(name="u_ps", bufs=1, space="PSUM") as u_ps,
                    tc.tile_pool(name="sps", bufs=2, space="PSUM") as sps,
                    tc.tile_pool(name="usb_pool", bufs=1) as usb_pool,
                    tc.tile_pool(name="misc", bufs=1) as misc,
                ):
                    for ns in range(0 if b_off else nsl_eff):
                        if ns + 2 < nsl_eff:
                            fresh_xt(ns + 2, "b")
                        # project this slice's q strip (transposed layout)
                        qsbuf = []
                        for hc in range(ND):
                            ps = sps.tile([P, SL], F32, name="s_ps", tag="s")
                            mm_group(
                                ps, [wslice(wq, c, hc) for c in range(ND)],
                                lambda c, lo, w, _ns=ns: xts[_ns][c][:, lo : lo + w],
                                split=split,
                            )
                            qc = qsb_pool.tile([P, SL], F32R, name=f"qsb{hc}", tag=f"qsb{hc}")
                            nc.scalar.activation(
                                qc[:], ps[:], AF.Identity, bias=bcol_sb[:, hc : hc + 1]
                            )
                            qsbuf.append(qc)
                        us = [
                            u_ps.tile([P, SL], F32, name=f"u{c}", tag=f"u{c}")
                            for c in range(ND)
                        ]
                        r_sb = misc.tile([P, SL], F32, name="r_sb", tag="r_sb")

                        p_sbs = [None] * NM
                        for mi in range(nm_eff + 1):
                            if mi < nm_eff:
                                msl = slice(mi * P, (mi + 1) * P)
                                s_ps = sps.tile([P, SL], F32, name="s_ps", tag="s")
                                mm_group(
                                    s_ps, [ktsb[c][:, msl] for c in range(ND)],
                                    lambda c, lo, w: qsbuf[c][:, lo : lo + w],
                                    split=split,
                                )
                                p_sb = p_pool.tile([P, SL], pdt, name="p_sb", tag="p")
                                nc.scalar.activation(
                                    p_sb[:], s_ps[:], AF.Copy if copy_exp else AF.Exp
                                )
                                p_sbs[mi] = p_sb
                            if mi >= 1:
                                j = mi - 1
                                pj = p_sbs[j]
                                if not no_r:
                                    r_ps = sps.tile([P, SL], F32, name="r_ps", tag="s")
                                    mm_group(
                                        r_ps, [ones128[:]],
                                        lambda c, lo, w: pj[:, lo : lo + w],
                                        split=split,
                                    )
                                    if j == 0:
                                        nc.vector.tensor_copy(r_sb[:], r_ps[:])
                                    else:
                                        nc.vector.tensor_tensor(
                                            r_sb[:], r_ps[:], r_sb[:], op=ALU.add
                                        )
                                for c in range(ND):
                                    hw2 = SL // split
                                    for h in range(split):
                                        lo = h * hw2
                                        nc.tensor.matmul(
                                            us[c][:, lo : lo + hw2],
                                            vsb[j][:, c * P : (c + 1) * P],
                                            pj[:, lo : lo + hw2],
                                            start=(j == 0 and h == 0),
                                            stop=(j == nm_eff - 1 and h == split - 1),
                                        )
                                p_sbs[j] = None

                        rinv = misc.tile([P, SL], F32, name="rinv", tag="rinv")
                        if no_r:
                            nc.vector.tensor_copy(rinv[:], r_sb[:])
                        else:
                            nc.vector.reciprocal(rinv[:], r_sb[:])

                        for c in range(ND):
                            u_sb = usb_pool.tile([P, SL], F32, name=f"usb{c}", tag=f"usb{c}")
                            nc.vector.tensor_tensor(u_sb[:], us[c][:], rinv[:], op=ALU.mult)
                            store_eng = nc.gpsimd if STORE_GP else nc.sync
                            store_eng.dma_start(
                                out.ap()[c * P : (c + 1) * P, ns * SL : (ns + 1) * SL],
                                u_sb[:],
                            )

    nc.compile()
    return nc


_NC = None


def kernel(x: np.ndarray, W: np.ndarray, b: np.ndarray) -> np.ndarray:
    global _NC
    if _NC is None:
        _NC = build_nc()

    x = np.ascontiguousarray(x, dtype=np.float32)
    W = np.ascontiguousarray(W, dtype=np.float32)
    b = np.ascontiguousarray(b, dtype=np.float32)

    wT = np.ascontiguousarray(W.T)                      # [768, 2304]
    bcol = np.ascontiguousarray(b[: 2 * H].reshape(2 * ND, P).T)  # [128, 12]
    bvrow = np.ascontiguousarray(b[2 * H :].reshape(1, H))

    in_maps = []
    for i in range(B):
        in_maps.append(
            {
                "xT": np.ascontiguousarray(x[i].T),     # [768, 2048]
                "wT": wT,
                "bcol": bcol,
                "bvrow": bvrow,
            }
        )

    res = run_bass_kernel_spmd(_NC, in_maps, core_ids=list(range(B)))
    return np.stack(
        [np.ascontiguousarray(res.results[i]["out"].T) for i in range(B)], axis=0
    )

